# revision 10
# baseline (speedup 1.0000x reference)
"""Cross-attention Trainium2 kernel (Bass/Tile), data-parallel over batch.

B=8 batch elements -> 8 NeuronCores, one batch element per core.
Per core: y = softmax(q Wq (kv Wk)^T / sqrt(dk)) (kv Wv) Wo + bo
with S1=S2=2048, D=1024, H=8, DK=DV=128.

v3 (869us baseline -> 612us v2 -> this):
  - all loads via HWDGE (RTL descriptor generation); f32 staged then
    cast to bf16 on gpsimd/DVE. v2 used SWDGE cast-DMAs whose Q7
    descriptor generation (~46us per 2MB input block) starved the PE
    at startup and between kv blocks.
  - input transposes on the DMA xbar (dma_start_transpose).
  - Q projection interleaved into the kv phase (fills PE stalls);
    attention phase is then pure scores/softmax/PV/out-proj.
  - softmax row-sums: bf16 add-tree on DVE + one ones-matmul;
    reciprocal_approx_fast; gpsimd partition_broadcast.
  - output bias folded into the PSUM eviction as a DVE tensor_add.
  - scores PSUM [128,2,512] double-buffered; PV PSUM double-buffered;
    head loop software-pipelined (PE: scores(h) then rowsum+PV(h-1)).
"""

import os

import numpy as np

import concourse.bass as bass
import concourse.mybir as mybir
import concourse.tile as tile
from concourse import bacc
from concourse.bass_utils import run_bass_kernel_spmd

B = 8
S = 2048  # S1 == S2
D = 1024  # D1 == D2
H = 8
DK = DV = 128
KC = D // 128  # contraction chunks of the model dim
SC = S // 128  # sequence chunks of 128
BLK = 512
NBLK = S // BLK
SCALE = 1.0 / float(np.sqrt(DK))

F32 = mybir.dt.float32
BF16 = mybir.dt.bfloat16
EXP = mybir.ActivationFunctionType.Exp


def _emit(tc, aps):
    nc = tc.nc
    query, key_value, Wq, Wk, Wv, Wo, bo, out = (
        aps["query"], aps["key_value"], aps["Wq"], aps["Wk"], aps["Wv"],
        aps["Wo"], aps["bo"], aps["out"],
    )

    persist = tc.alloc_tile_pool(name="persist", bufs=1)
    KT_sb = persist.tile([128, H, S], BF16, name="KT_sb")
    V_sb = persist.tile([128, SC, H * DV], BF16, name="V_sb")
    Wo_sb = persist.tile([128, KC, D], BF16, name="Wo_sb")
    bo_bc = persist.tile([128, D], F32, name="bo_bc")
    onec_sb = persist.tile([128, 1], BF16, name="onec_sb")
    QT = persist.tile([128, NBLK, H, BLK], BF16, name="QT_sb")
    nc.vector.memset(onec_sb, 1.0)

    # bias broadcast [1,D] -> [128,D] straight from DRAM via stride-0 AP
    bo_bcast_src = bass.AP(
        tensor=bo.tensor, offset=bo.offset,
        ap=[[0, 128]] + list(bo.ap[1:]),
    )
    nc.sync.dma_start(out=bo_bc, in_=bo_bcast_src)

    # ---- phase 1: load/transpose inputs, project K/V and Q -------------
    with nc.named_scope("ph1"), \
         tc.tile_pool(name="p1w", bufs=1) as p1, \
         tc.tile_pool(name="p1psum", bufs=4, space="PSUM") as pps, \
         tc.tile_pool(name="qpsum", bufs=1, space="PSUM") as qpp:
        Wk_sb = p1.tile([128, KC, D], BF16, name="Wk_sb")
        Wv_sb = p1.tile([128, KC, D], BF16, name="Wv_sb")
        Wq_sb = p1.tile([128, KC, D], BF16, name="Wq_sb")

        def load_weight(dst, src, cast_engine):
            # HWDGE f32 load per 128-row chunk, engine cast to bf16
            srcv = src.rearrange("(kc p) n -> p kc n", p=128)
            for kc in range(KC):
                ws = p1.tile([128, D], F32, name="wstage", tag="wstage",
                             bufs=2)
                nc.sync.dma_start(out=ws, in_=srcv[:, kc, :])
                cast_engine.tensor_copy(dst[:, kc, :], ws)

        def load_block_T(src_ap, j, tag, cast_engine):
            """512 rows of src [S, D] f32 -> bf16 xT [128, 4, KC, 128]
            (partition = col within chunk, s-subchunk, chunk, row)."""
            xn = p1.tile([128, 4, D], BF16, name=f"{tag}n", tag="xn",
                         bufs=1)
            srcv = src_ap[j * BLK:(j + 1) * BLK, :].rearrange(
                "(c p) d -> p c d", p=128)
            for half in range(2):
                st = p1.tile([128, 2, D], F32, name=f"{tag}s", tag="xs",
                             bufs=1)
                nc.sync.dma_start(out=st, in_=srcv[:, 2 * half:2 * half + 2, :])
                cast_engine.tensor_copy(xn[:, 2 * half:2 * half + 2, :], st)
            xT = p1.tile([128, 4, KC, 128], BF16, name=f"{tag}T", tag="xT",
                         bufs=2)
            for c4 in range(4):
                nc.sync.dma_start_transpose(out=xT[:, c4], in_=xn[:, c4, :])
            return xT

        first = True
        for j in range(NBLK):
            kvT = load_block_T(key_value, j, "kv", nc.gpsimd)
            if first:
                load_weight(Wk_sb, Wk, nc.gpsimd)
                load_weight(Wv_sb, Wv, nc.gpsimd)
                load_weight(Wq_sb, Wq, nc.gpsimd)
            # KT block: out[dk(h), s2-block] += Wk[kc,h].T @ kvT[kc]
            for h in range(H):
                ps = pps.tile([128, BLK], F32, name="ps_k", tag="pps")
                for kc in range(KC):
                    nc.tensor.matmul(
                        ps, lhsT=Wk_sb[:, kc, h * 128:(h + 1) * 128],
                        rhs=kvT[:, :, kc, :], start=(kc == 0),
                        stop=(kc == KC - 1),
                    )
                nc.scalar.copy(KT_sb[:, h, j * BLK:(j + 1) * BLK], ps)
            # V rows: out[s2 sub, hdv-block] += kvT[kc, sub].T @ Wv[kc]
            for m4 in range(4):
                for n in range(2):
                    ps = pps.tile([128, BLK], F32, name="ps_v", tag="pps")
                    for kc in range(KC):
                        nc.tensor.matmul(
                            ps, lhsT=kvT[:, m4, kc, :],
                            rhs=Wv_sb[:, kc, n * BLK:(n + 1) * BLK],
                            start=(kc == 0), stop=(kc == KC - 1),
                        )
                    nc.vector.tensor_copy(
                        V_sb[:, j * 4 + m4, n * BLK:(n + 1) * BLK], ps
                    )
            # Q block: load/transpose/project interleaved with kv work
            qT = load_block_T(query, j, "q", nc.vector)
            if first:
                load_weight(Wo_sb, Wo, nc.gpsimd)
                first = False
            for h in range(H):
                ps = qpp.tile([128, BLK], F32, name="ps_q", tag="qps")
                for kc in range(KC):
                    nc.tensor.matmul(
                        ps, lhsT=Wq_sb[:, kc, h * 128:(h + 1) * 128],
                        rhs=qT[:, :, kc, :], start=(kc == 0),
                        stop=(kc == KC - 1),
                    )
                nc.vector.tensor_copy(QT[:, j, h, :], ps)

    # ---- phase 2: attention + output projection ------------------------
    with nc.named_scope("attn"), \
         tc.tile_pool(name="p2", bufs=1) as p2, \
         tc.tile_pool(name="scpsum", bufs=2, space="PSUM") as scp, \
         tc.tile_pool(name="pvpsum", bufs=2, space="PSUM") as pvp, \
         tc.tile_pool(name="ypsum", bufs=1, space="PSUM") as ypp, \
         tc.tile_pool(name="supsum", bufs=1, space="PSUM") as sup:

        def head_tail(h, PT_h, OT_sb):
            """rowsum finish + PV + normalize for head h."""
            t1 = p2.tile([128, 8, BLK], BF16, name="t1", tag="t1")
            nc.vector.tensor_add(t1, PT_h[:, 0:8, :], PT_h[:, 8:16, :])
            t2 = p2.tile([128, 4, BLK], BF16, name="t2", tag="t2")
            nc.vector.tensor_add(t2, t1[:, 0:4, :], t1[:, 4:8, :])
            t3 = p2.tile([128, 2, BLK], BF16, name="t3", tag="t3")
            nc.vector.tensor_add(t3, t2[:, 0:2, :], t2[:, 2:4, :])
            t4 = p2.tile([128, BLK], BF16, name="t4", tag="t4")
            nc.vector.tensor_add(t4, t3[:, 0, :], t3[:, 1, :])
            sus = sup.tile([1, BLK], F32, name="sus", tag="sus")
            nc.tensor.matmul(sus, lhsT=onec_sb, rhs=t4, start=True,
                             stop=True)
            rec = p2.tile([1, BLK], F32, name="rec", tag="rec", bufs=1)
            nc.vector.reciprocal_approx_fast(out=rec, in_=sus)
            bc = p2.tile([128, BLK], F32, name="bc", tag="bc", bufs=2)
            nc.gpsimd.partition_broadcast(bc, rec)
            # PV: OT[dv, s1] += V_c[:,h].T @ PT_c
            ops = pvp.tile([128, BLK], F32, name="ops", tag="ops")
            for c in range(SC):
                nc.tensor.matmul(
                    ops, lhsT=V_sb[:, c, h * 128:(h + 1) * 128],
                    rhs=PT_h[:, c, :], start=(c == 0), stop=(c == SC - 1),
                )
            nc.vector.tensor_mul(OT_sb[:, h, :], ops, bc)

        for j in range(NBLK):
            OT_sb = p2.tile([128, H, BLK], BF16, name="OT_sb", tag="OT",
                            bufs=2)
            prev = None
            for h in range(H):
                PT_h = p2.tile([128, SC, BLK], BF16, name="PT", tag="PT",
                               bufs=2)
                qblk = QT[:, j, h, :]
                for g in range(8):
                    sps = scp.tile([128, 2, BLK], F32, name="sps",
                                   tag="sps")
                    for i in range(2):
                        c = 2 * g + i
                        nc.tensor.matmul(
                            sps[:, i, :],
                            lhsT=KT_sb[:, h, c * 128:(c + 1) * 128],
                            rhs=qblk, start=True, stop=True,
                        )
                    nc.scalar.activation(
                        PT_h[:, 2 * g:2 * g + 2, :], sps, EXP, scale=SCALE
                    )
                if prev is not None:
                    head_tail(prev[0], prev[1], OT_sb)
                prev = (h, PT_h)
            head_tail(prev[0], prev[1], OT_sb)

            # output projection for block j (bias via DVE add)
            for m in range(4):
                for n in range(2):
                    yps = ypp.tile([128, BLK], F32, name="yps", tag="yps")
                    for h in range(H):
                        nc.tensor.matmul(
                            yps, lhsT=OT_sb[:, h, m * 128:(m + 1) * 128],
                            rhs=Wo_sb[:, h, n * BLK:(n + 1) * BLK],
                            start=(h == 0), stop=(h == H - 1),
                        )
                    y_sb = p2.tile([128, BLK], F32, name="y_sb", tag="y",
                                   bufs=2)
                    nc.vector.tensor_add(
                        y_sb, yps, bo_bc[:, n * BLK:(n + 1) * BLK]
                    )
                    r0 = j * BLK + m * 128
                    nc.scalar.dma_start(
                        out=out[r0:r0 + 128, n * BLK:(n + 1) * BLK],
                        in_=y_sb,
                    )
    persist.release()


_CACHE = {}


def _build():
    if "nc" in _CACHE:
        return _CACHE["nc"]
    nc = bacc.Bacc(
        "TRN2", target_bir_lowering=False, debug=False,
        enable_asserts=False, num_devices=B,
    )
    aps = {
        "query": nc.dram_tensor("query", [S, D], F32, kind="ExternalInput").ap(),
        "key_value": nc.dram_tensor("key_value", [S, D], F32, kind="ExternalInput").ap(),
        "Wq": nc.dram_tensor("Wq", [D, H * DK], F32, kind="ExternalInput").ap(),
        "Wk": nc.dram_tensor("Wk", [D, H * DK], F32, kind="ExternalInput").ap(),
        "Wv": nc.dram_tensor("Wv", [D, H * DV], F32, kind="ExternalInput").ap(),
        "Wo": nc.dram_tensor("Wo", [H * DV, D], F32, kind="ExternalInput").ap(),
        "bo": nc.dram_tensor("bo", [1, D], F32, kind="ExternalInput").ap(),
        "out": nc.dram_tensor("out", [S, D], F32, kind="ExternalOutput").ap(),
    }
    with tile.TileContext(nc) as tc:
        _emit(tc, aps)
    nc.compile()
    _CACHE["nc"] = nc
    return nc


LAST_RESULT = None


def kernel(query, key_value, Wq, Wk, Wv, Wo, bo):
    global LAST_RESULT
    nc = _build()
    query = np.ascontiguousarray(np.asarray(query, dtype=np.float32))
    key_value = np.ascontiguousarray(np.asarray(key_value, dtype=np.float32))
    shared = {
        "Wq": np.ascontiguousarray(np.asarray(Wq, dtype=np.float32)),
        "Wk": np.ascontiguousarray(np.asarray(Wk, dtype=np.float32)),
        "Wv": np.ascontiguousarray(np.asarray(Wv, dtype=np.float32)),
        "Wo": np.ascontiguousarray(np.asarray(Wo, dtype=np.float32)),
        "bo": np.ascontiguousarray(np.asarray(bo, dtype=np.float32)).reshape(1, D),
    }
    in_maps = [
        {"query": query[i], "key_value": key_value[i], **shared} for i in range(B)
    ]
    res = run_bass_kernel_spmd(
        nc, in_maps, core_ids=list(range(B)),
        trace=bool(int(os.environ.get("KERNEL_TRACE", "0"))),
    )
    LAST_RESULT = res
    return np.stack([r["out"] for r in res.results]).astype(np.float32)


if __name__ == "__main__":
    rng = np.random.default_rng(0)
    inputs = {
        "query": rng.standard_normal((B, S, D), dtype=np.float32),
        "key_value": rng.standard_normal((B, S, D), dtype=np.float32),
        "Wq": (rng.random((D, H * DK), dtype=np.float32) - 0.5) / 16.0,
        "Wk": (rng.random((D, H * DK), dtype=np.float32) - 0.5) / 16.0,
        "Wv": (rng.random((D, H * DV), dtype=np.float32) - 0.5) / 16.0,
        "Wo": (rng.random((H * DV, D), dtype=np.float32) - 0.5) / 16.0,
        "bo": (rng.random(D, dtype=np.float32) - 0.5) / 16.0,
    }
    y = kernel(**inputs)
    print("kernel out", y.shape, y.dtype, float(np.abs(y).max()))


# revision 13
# speedup vs baseline: 1.0925x; 1.0925x over previous
"""Cross-attention Trainium2 kernel (Bass/Tile), data-parallel over batch.

B=8 batch elements -> 8 NeuronCores, one batch element per core.
Per core: y = softmax(q Wq (kv Wk)^T / sqrt(dk)) (kv Wv) Wo + bo
with S1=S2=2048, D=1024, H=8, DK=DV=128.

v3 (869us baseline -> 612us v2 -> this):
  - all loads via HWDGE (RTL descriptor generation); f32 staged then
    cast to bf16 on gpsimd/DVE. v2 used SWDGE cast-DMAs whose Q7
    descriptor generation (~46us per 2MB input block) starved the PE
    at startup and between kv blocks.
  - input transposes on the DMA xbar (dma_start_transpose).
  - Q projection interleaved into the kv phase (fills PE stalls);
    attention phase is then pure scores/softmax/PV/out-proj.
  - softmax row-sums: bf16 add-tree on DVE + one ones-matmul;
    reciprocal_approx_fast; gpsimd partition_broadcast.
  - output bias folded into the PSUM eviction as a DVE tensor_add.
  - scores PSUM [128,2,512] double-buffered; PV PSUM double-buffered;
    head loop software-pipelined (PE: scores(h) then rowsum+PV(h-1)).
"""

import os

import numpy as np

import concourse.bass as bass
import concourse.mybir as mybir
import concourse.tile as tile
from concourse import bacc
from concourse.bass_utils import run_bass_kernel_spmd

B = 8
S = 2048  # S1 == S2
D = 1024  # D1 == D2
H = 8
DK = DV = 128
KC = D // 128  # contraction chunks of the model dim
SC = S // 128  # sequence chunks of 128
BLK = 512
NBLK = S // BLK
SCALE = 1.0 / float(np.sqrt(DK))

F32 = mybir.dt.float32
BF16 = mybir.dt.bfloat16
EXP = mybir.ActivationFunctionType.Exp


def _emit(tc, aps):
    nc = tc.nc
    query, key_value, Wq, Wk, Wv, Wo, bo, out = (
        aps["query"], aps["key_value"], aps["Wq"], aps["Wk"], aps["Wv"],
        aps["Wo"], aps["bo"], aps["out"],
    )

    persist = tc.alloc_tile_pool(name="persist", bufs=1)
    KT_sb = persist.tile([128, H, S], BF16, name="KT_sb")
    V_sb = persist.tile([128, SC, H * DV], BF16, name="V_sb")
    Wo_sb = persist.tile([128, KC, D], BF16, name="Wo_sb")
    bo_bc = persist.tile([128, D], F32, name="bo_bc")
    onec_sb = persist.tile([128, 1], BF16, name="onec_sb")
    QT = persist.tile([128, NBLK, H, BLK], BF16, name="QT_sb")
    nc.vector.memset(onec_sb, 1.0)

    # bias broadcast [1,D] -> [128,D] straight from DRAM via stride-0 AP
    bo_bcast_src = bass.AP(
        tensor=bo.tensor, offset=bo.offset,
        ap=[[0, 128]] + list(bo.ap[1:]),
    )
    nc.sync.dma_start(out=bo_bc, in_=bo_bcast_src)

    # ---- phase 1: load/transpose inputs, project K/V and Q -------------
    with nc.named_scope("ph1"), \
         tc.tile_pool(name="p1w", bufs=1) as p1, \
         tc.tile_pool(name="p1psum", bufs=4, space="PSUM") as pps, \
         tc.tile_pool(name="qpsum", bufs=1, space="PSUM") as qpp:
        Wk_sb = p1.tile([128, KC, D], BF16, name="Wk_sb")
        Wv_sb = p1.tile([128, KC, D], BF16, name="Wv_sb")
        Wq_sb = p1.tile([128, KC, D], BF16, name="Wq_sb")

        def load_weight(dst, src, cast_engine):
            # HWDGE f32 load per 128-row chunk, engine cast to bf16
            srcv = src.rearrange("(kc p) n -> p kc n", p=128)
            for kc in range(KC):
                ws = p1.tile([128, D], F32, name="wstage", tag="wstage",
                             bufs=2)
                nc.sync.dma_start(out=ws, in_=srcv[:, kc, :])
                cast_engine.tensor_copy(dst[:, kc, :], ws)

        def load_block_T(src_ap, j, tag, cast_engine):
            """512 rows of src [S, D] f32 -> bf16 xT [128, 4, KC, 128]
            (partition = col within chunk, s-subchunk, chunk, row)."""
            xn = p1.tile([128, 4, D], BF16, name=f"{tag}n", tag="xn",
                         bufs=1)
            srcv = src_ap[j * BLK:(j + 1) * BLK, :].rearrange(
                "(c p) d -> p c d", p=128)
            for half in range(2):
                st = p1.tile([128, 2, D], F32, name=f"{tag}s", tag="xs",
                             bufs=1)
                nc.sync.dma_start(out=st, in_=srcv[:, 2 * half:2 * half + 2, :])
                cast_engine.tensor_copy(xn[:, 2 * half:2 * half + 2, :], st)
            xT = p1.tile([128, 4, KC, 128], BF16, name=f"{tag}T", tag="xT",
                         bufs=2)
            for c4 in range(4):
                nc.sync.dma_start_transpose(out=xT[:, c4], in_=xn[:, c4, :])
            return xT

        first = True
        for j in range(NBLK):
            kvT = load_block_T(key_value, j, "kv", nc.vector)
            if first:
                load_weight(Wk_sb, Wk, nc.vector)
                load_weight(Wv_sb, Wv, nc.vector)
                load_weight(Wq_sb, Wq, nc.vector)
            # KT block: out[dk(h), s2-block] += Wk[kc,h].T @ kvT[kc]
            for h in range(H):
                ps = pps.tile([128, BLK], F32, name="ps_k", tag="pps")
                for kc in range(KC):
                    nc.tensor.matmul(
                        ps, lhsT=Wk_sb[:, kc, h * 128:(h + 1) * 128],
                        rhs=kvT[:, :, kc, :], start=(kc == 0),
                        stop=(kc == KC - 1),
                    )
                nc.scalar.copy(KT_sb[:, h, j * BLK:(j + 1) * BLK], ps)
            # V rows: out[s2 sub, hdv-block] += kvT[kc, sub].T @ Wv[kc]
            for m4 in range(4):
                for n in range(2):
                    ps = pps.tile([128, BLK], F32, name="ps_v", tag="pps")
                    for kc in range(KC):
                        nc.tensor.matmul(
                            ps, lhsT=kvT[:, m4, kc, :],
                            rhs=Wv_sb[:, kc, n * BLK:(n + 1) * BLK],
                            start=(kc == 0), stop=(kc == KC - 1),
                        )
                    nc.vector.tensor_copy(
                        V_sb[:, j * 4 + m4, n * BLK:(n + 1) * BLK], ps
                    )
            # Q block: load/transpose/project interleaved with kv work
            qT = load_block_T(query, j, "q", nc.vector)
            if first:
                load_weight(Wo_sb, Wo, nc.vector)
                first = False
            for h in range(H):
                ps = qpp.tile([128, BLK], F32, name="ps_q", tag="qps")
                for kc in range(KC):
                    nc.tensor.matmul(
                        ps, lhsT=Wq_sb[:, kc, h * 128:(h + 1) * 128],
                        rhs=qT[:, :, kc, :], start=(kc == 0),
                        stop=(kc == KC - 1),
                    )
                nc.vector.tensor_copy(QT[:, j, h, :], ps)

    # ---- phase 2: attention + output projection ------------------------
    with nc.named_scope("attn"), \
         tc.tile_pool(name="p2", bufs=1) as p2, \
         tc.tile_pool(name="scpsum", bufs=2, space="PSUM") as scp, \
         tc.tile_pool(name="pvpsum", bufs=2, space="PSUM") as pvp, \
         tc.tile_pool(name="ypsum", bufs=1, space="PSUM") as ypp, \
         tc.tile_pool(name="supsum", bufs=1, space="PSUM") as sup:

        def head_tail(h, PT_h, OT_sb):
            """rowsum finish + PV + normalize for head h."""
            t1 = p2.tile([128, 8, BLK], BF16, name="t1", tag="t1")
            nc.vector.tensor_add(t1, PT_h[:, 0:8, :], PT_h[:, 8:16, :])
            t2 = p2.tile([128, 4, BLK], BF16, name="t2", tag="t2")
            nc.vector.tensor_add(t2, t1[:, 0:4, :], t1[:, 4:8, :])
            t3 = p2.tile([128, 2, BLK], BF16, name="t3", tag="t3")
            nc.vector.tensor_add(t3, t2[:, 0:2, :], t2[:, 2:4, :])
            t4 = p2.tile([128, BLK], BF16, name="t4", tag="t4")
            nc.vector.tensor_add(t4, t3[:, 0, :], t3[:, 1, :])
            # PV: OT[dv, s1] += V_c[:,h].T @ PT_c  (before the rowsum
            # matmul so the PE never waits on the DVE tree)
            ops = pvp.tile([128, BLK], F32, name="ops", tag="ops")
            for c in range(SC):
                nc.tensor.matmul(
                    ops, lhsT=V_sb[:, c, h * 128:(h + 1) * 128],
                    rhs=PT_h[:, c, :], start=(c == 0), stop=(c == SC - 1),
                )
            sus = sup.tile([1, BLK], F32, name="sus", tag="sus")
            nc.tensor.matmul(sus, lhsT=onec_sb, rhs=t4, start=True,
                             stop=True)
            rec = p2.tile([1, BLK], F32, name="rec", tag="rec", bufs=1)
            nc.vector.reciprocal_approx_fast(out=rec, in_=sus)
            bc = p2.tile([128, BLK], F32, name="bc", tag="bc", bufs=2)
            nc.gpsimd.partition_broadcast(bc, rec)
            nc.vector.tensor_mul(OT_sb[:, h, :], ops, bc)

        for j in range(NBLK):
            OT_sb = p2.tile([128, H, BLK], BF16, name="OT_sb", tag="OT",
                            bufs=2)
            prev = None
            for h in range(H):
                PT_h = p2.tile([128, SC, BLK], BF16, name="PT", tag="PT",
                               bufs=2)
                qblk = QT[:, j, h, :]
                for g in range(8):
                    sps = scp.tile([128, 2, BLK], F32, name="sps",
                                   tag="sps")
                    for i in range(2):
                        c = 2 * g + i
                        nc.tensor.matmul(
                            sps[:, i, :],
                            lhsT=KT_sb[:, h, c * 128:(c + 1) * 128],
                            rhs=qblk, start=True, stop=True,
                        )
                    nc.scalar.activation(
                        PT_h[:, 2 * g:2 * g + 2, :], sps, EXP, scale=SCALE
                    )
                if prev is not None:
                    head_tail(prev[0], prev[1], OT_sb)
                prev = (h, PT_h)
            head_tail(prev[0], prev[1], OT_sb)

            # output projection for block j (bias via DVE add)
            for m in range(4):
                for n in range(2):
                    yps = ypp.tile([128, BLK], F32, name="yps", tag="yps")
                    for h in range(H):
                        nc.tensor.matmul(
                            yps, lhsT=OT_sb[:, h, m * 128:(m + 1) * 128],
                            rhs=Wo_sb[:, h, n * BLK:(n + 1) * BLK],
                            start=(h == 0), stop=(h == H - 1),
                        )
                    y_sb = p2.tile([128, BLK], F32, name="y_sb", tag="y",
                                   bufs=2)
                    nc.vector.tensor_add(
                        y_sb, yps, bo_bc[:, n * BLK:(n + 1) * BLK]
                    )
                    r0 = j * BLK + m * 128
                    nc.scalar.dma_start(
                        out=out[r0:r0 + 128, n * BLK:(n + 1) * BLK],
                        in_=y_sb,
                    )
    persist.release()


_CACHE = {}


def _build():
    if "nc" in _CACHE:
        return _CACHE["nc"]
    nc = bacc.Bacc(
        "TRN2", target_bir_lowering=False, debug=False,
        enable_asserts=False, num_devices=B,
    )
    aps = {
        "query": nc.dram_tensor("query", [S, D], F32, kind="ExternalInput").ap(),
        "key_value": nc.dram_tensor("key_value", [S, D], F32, kind="ExternalInput").ap(),
        "Wq": nc.dram_tensor("Wq", [D, H * DK], F32, kind="ExternalInput").ap(),
        "Wk": nc.dram_tensor("Wk", [D, H * DK], F32, kind="ExternalInput").ap(),
        "Wv": nc.dram_tensor("Wv", [D, H * DV], F32, kind="ExternalInput").ap(),
        "Wo": nc.dram_tensor("Wo", [H * DV, D], F32, kind="ExternalInput").ap(),
        "bo": nc.dram_tensor("bo", [1, D], F32, kind="ExternalInput").ap(),
        "out": nc.dram_tensor("out", [S, D], F32, kind="ExternalOutput").ap(),
    }
    with tile.TileContext(nc) as tc:
        _emit(tc, aps)
    nc.compile()
    _CACHE["nc"] = nc
    return nc


LAST_RESULT = None


def kernel(query, key_value, Wq, Wk, Wv, Wo, bo):
    global LAST_RESULT
    nc = _build()
    query = np.ascontiguousarray(np.asarray(query, dtype=np.float32))
    key_value = np.ascontiguousarray(np.asarray(key_value, dtype=np.float32))
    shared = {
        "Wq": np.ascontiguousarray(np.asarray(Wq, dtype=np.float32)),
        "Wk": np.ascontiguousarray(np.asarray(Wk, dtype=np.float32)),
        "Wv": np.ascontiguousarray(np.asarray(Wv, dtype=np.float32)),
        "Wo": np.ascontiguousarray(np.asarray(Wo, dtype=np.float32)),
        "bo": np.ascontiguousarray(np.asarray(bo, dtype=np.float32)).reshape(1, D),
    }
    in_maps = [
        {"query": query[i], "key_value": key_value[i], **shared} for i in range(B)
    ]
    res = run_bass_kernel_spmd(
        nc, in_maps, core_ids=list(range(B)),
        trace=bool(int(os.environ.get("KERNEL_TRACE", "0"))),
    )
    LAST_RESULT = res
    return np.stack([r["out"] for r in res.results]).astype(np.float32)


if __name__ == "__main__":
    rng = np.random.default_rng(0)
    inputs = {
        "query": rng.standard_normal((B, S, D), dtype=np.float32),
        "key_value": rng.standard_normal((B, S, D), dtype=np.float32),
        "Wq": (rng.random((D, H * DK), dtype=np.float32) - 0.5) / 16.0,
        "Wk": (rng.random((D, H * DK), dtype=np.float32) - 0.5) / 16.0,
        "Wv": (rng.random((D, H * DV), dtype=np.float32) - 0.5) / 16.0,
        "Wo": (rng.random((H * DV, D), dtype=np.float32) - 0.5) / 16.0,
        "bo": (rng.random(D, dtype=np.float32) - 0.5) / 16.0,
    }
    y = kernel(**inputs)
    print("kernel out", y.shape, y.dtype, float(np.abs(y).max()))
